# revision 1
# baseline (speedup 1.0000x reference)
"""AnchorTriangleAttention distributed across 8 Trainium2 NeuronCores.

Sharding (per spec hint): row-parallel over the first residue axis i.
Each core owns Li = L/8 = 64 rows of pair_repr. The anchor-row gather
(pair_row, K=32 rows) and the anchor-column template slices are
replicated to every core; weights are replicated. All gathers
(anchor_idx fancy-indexing) happen on the host; each core then runs a
dense gather-free graph: 5 projections, anchor-triangle scores +
template bias, softmax over K, value contraction, output projection,
and the sigmoid-gated residual update.

Hardcoded shapes: B=1, L=512, K=32, D=128, A=64, SIGMA=4.0, 8 cores.
"""

import numpy as np

DIM = 128
ATTN_DIM = 64
K = 32
L = 512
B = 1
SIGMA = 4.0
N_CORES = 8
LI = L // N_CORES  # 64 rows of i per core


def _template_gate_host(template_dist, template_quality, Tg_W1, Tg_b1, Tg_W2, Tg_b2):
    """Tiny scalar gate MLP — replicates reference._template_gate in numpy."""
    td = np.asarray(template_dist, dtype=np.float32)
    mask = (td > 0).astype(np.float32)
    coverage = mask.mean(axis=(1, 2))  # [B]
    length = td.shape[-1]
    length_norm = np.full_like(coverage, length / 512.0)
    feats = np.stack(
        [coverage, np.asarray(template_quality, np.float32), length_norm], axis=-1
    )  # [B,3]
    h = np.maximum(feats @ np.asarray(Tg_W1, np.float32) + np.asarray(Tg_b1, np.float32), 0.0)
    z = h @ np.asarray(Tg_W2, np.float32) + np.asarray(Tg_b2, np.float32)
    gate = 1.0 / (1.0 + np.exp(-z))  # [B,1]
    return gate.reshape(-1)  # [B]


def _build_shard_fn():
    import jax
    import jax.numpy as jnp

    def shard_fn(x, xa, xr, t_i, t_l, t_r, gscale, Wq, Wl, Wr, Wvl, Wvr, Wo, Wg, bg):
        # x:   [Li, L, D]   this core's rows of pair_repr
        # xa:  [Li, K, D]   pair_col shard  (host gather over anchors)
        # xr:  [K, L, D]    pair_row        (replicated anchor rows)
        # t_i: [Li, L]      template_dist rows
        # t_l: [Li, K]      template_dist rows at anchor cols
        # t_r: [L, K]       template_dist anchor rows, transposed
        # gscale: [1]       gate / SIGMA (host-computed scalar)
        q = jnp.einsum("ijd,da->ija", x, Wq)
        left = jnp.einsum("ikd,da->ika", xa, Wl)
        right = jnp.einsum("kjd,da->kja", xr, Wr)

        scores = jnp.einsum("ija,ika->ijk", q, left)
        scores = scores + jnp.einsum("ija,kja->ijk", q, right)
        scores = scores * (1.0 / np.sqrt(np.float32(ATTN_DIM)))

        t_sum = t_l[:, None, :] + t_r[None, :, :]          # [Li, L, K]
        bias = -jnp.abs(t_sum - t_i[..., None]) * gscale   # gate/SIGMA folded
        scores = scores + bias

        attn = jax.nn.softmax(scores, axis=-1)             # [Li, L, K]

        v_left = jnp.einsum("ikd,da->ika", xa, Wvl)
        v_right = jnp.einsum("kjd,da->kja", xr, Wvr)

        up = jnp.einsum("ijk,ika->ija", attn, v_left)
        up = up + jnp.einsum("ijk,kja->ija", attn, v_right)
        up = jnp.einsum("ija,ad->ijd", up, Wo)

        g = jax.nn.sigmoid(jnp.einsum("ijd,de->ije", x, Wg) + bg)
        return x + g * up

    return shard_fn


def kernel(
    pair_repr,
    template_dist,
    template_quality,
    Wq,
    Wl,
    Wr,
    Wvl,
    Wvr,
    Wo,
    Wg,
    bg,
    Tg_W1,
    Tg_b1,
    Tg_W2,
    Tg_b2,
    anchor_idx,
):
    import jax

    devices = jax.devices()
    assert len(devices) >= N_CORES, f"need {N_CORES} cores, have {len(devices)}"
    devices = devices[:N_CORES]

    f32 = np.float32
    pr = np.asarray(pair_repr, f32)[0]        # [L, L, D]
    td = np.asarray(template_dist, f32)[0]    # [L, L]
    aidx = np.asarray(anchor_idx).astype(np.int64)

    gate = _template_gate_host(
        np.asarray(template_dist, f32),
        np.asarray(template_quality, f32),
        Tg_W1,
        Tg_b1,
        Tg_W2,
        Tg_b2,
    )  # [B]
    gscale = np.asarray([gate[0] / SIGMA], dtype=f32)

    # Host-side gathers (sanctioned by the sharding hint): anchors only.
    xr = np.ascontiguousarray(pr[aidx, :, :])        # [K, L, D] replicated
    t_r = np.ascontiguousarray(td[aidx, :].T)        # [L, K]    replicated

    weights = dict(
        Wq=np.asarray(Wq, f32),
        Wl=np.asarray(Wl, f32),
        Wr=np.asarray(Wr, f32),
        Wvl=np.asarray(Wvl, f32),
        Wvr=np.asarray(Wvr, f32),
        Wo=np.asarray(Wo, f32),
        Wg=np.asarray(Wg, f32),
        bg=np.asarray(bg, f32),
    )

    from jax.sharding import Mesh, NamedSharding, PartitionSpec as P

    mesh = Mesh(np.array(devices), ("x",))
    row = NamedSharding(mesh, P("x"))      # shard axis 0 over 8 cores
    rep = NamedSharding(mesh, P())         # replicated

    x = pr                                          # [L, L, D], shard rows
    xa = np.ascontiguousarray(pr[:, aidx, :])       # [L, K, D], shard rows
    t_i = td                                        # [L, L]
    t_l = np.ascontiguousarray(td[:, aidx])         # [L, K]

    shard_fn = _build_shard_fn()
    in_sh = (row, row, rep, row, row, rep, rep) + (rep,) * 8
    jitted = jax.jit(shard_fn, in_shardings=in_sh, out_shardings=row)

    args = (
        jax.device_put(x, row),
        jax.device_put(xa, row),
        jax.device_put(xr, rep),
        jax.device_put(t_i, row),
        jax.device_put(t_l, row),
        jax.device_put(t_r, rep),
        jax.device_put(gscale, rep),
        jax.device_put(weights["Wq"], rep),
        jax.device_put(weights["Wl"], rep),
        jax.device_put(weights["Wr"], rep),
        jax.device_put(weights["Wvl"], rep),
        jax.device_put(weights["Wvr"], rep),
        jax.device_put(weights["Wo"], rep),
        jax.device_put(weights["Wg"], rep),
        jax.device_put(weights["bg"], rep),
    )
    out = np.asarray(jitted(*args))  # [L, L, D]
    return out[None].astype(np.float32)  # [B, L, L, D]

